# revision 47
# baseline (speedup 1.0000x reference)
"""Trainium2 Bass kernel for nn_Attention_2 (8-head attention with positional bias).

Sharding: one head per NeuronCore (8 heads / 8 cores), data-parallel over the
full batch within each core.  Each core computes its head's projections,
attention (unnormalized softmax), and its partial contribution to the output
projection.  The host sums the 8 partial outputs.

v2 design notes:
- qh carries pre-scaled logits x' = (128/ln2) * scale * (q.Wq); kh plain.
- Per (ib, jg) tile, one of three elementwise paths turns sim PSUM into
  attention weights wt (bf16):
    'A': Act exp(ps/S) -> es, DVE mul es*exp(bias)[bf16]      (Act+DVE)
    'Y': PE identity-matmul adds S*bias into PSUM, Act exp(ps/S)  (PE+Act)
    'S': DVE (ps + ebS[i16]) -> int16, bitcast bf16 == Schraudolph exp
         with the bias and all constants folded into ebS        (DVE only)
  The tile counts balance the three engines; 'S' exploits bf16's bit layout
  (value ~= 2^((bits-16256)/128)) so one DVE op does exp+bias-mul.
- Sim matmuls run in 32-row PE groups; consecutive units alternate between
  row pairs (0-63 / 64-127) and the attn@v fifo pops in pairs so two units'
  sim matmuls are adjacent in program order and run concurrently.
- Out-projection in bf16 (4x fewer PE cycles than fp32); z stays f32 in PSUM
  until the single bf16 ot copy.
- DMAs batched: one qkv load per (b, ib), eb per half-ib sweep, one output
  store per (ib, b) into a p-major DRAM layout the host un-permutes.
"""

import os
import sys

sys.path.insert(0, "/opt/trn_rl_repo")

import numpy as np
import ml_dtypes
from contextlib import ExitStack

import concourse.bass as bass  # noqa: F401
import concourse.tile as tile
from concourse import bacc, mybir
from concourse.bass_utils import run_bass_kernel_spmd

B, N, D, H, DH = 4, 2048, 256, 8, 32
SCALE = DH ** -0.5
NCORES = 8
IB = 512            # i-block (query columns per matmul)
NIB = N // IB       # 4
JC = 128            # j-chunk (key rows per partition tile)
NJC = N // JC       # 16
JP = 2              # j-chunks packed per PE pass (row groups)
NJG = NJC // JP     # 8
F32 = mybir.dt.float32
BF16 = mybir.dt.bfloat16
I16 = mybir.dt.int16
AF = mybir.ActivationFunctionType
ALU = mybir.AluOpType

S_LOG2 = 128.0 / np.log(2.0)          # folds exp into bf16 bit layout
C_SCH = 16256.0 - 5.509 + float(os.environ.get("C_SCH_OFF", "0"))
N_S_TILES = int(os.environ.get("N_S_TILES", "18"))   # DVE Schraudolph tiles
N_Y_TILES = int(os.environ.get("N_Y_TILES", "8"))    # PE bias-add tiles
N_G_TILES = int(os.environ.get("N_G_TILES", "0"))    # Act exp + gpsimd mul tiles
FT_ACT = int(os.environ.get("FT_ACT", "1"))          # ft muls on Act engine
OT_ACT = int(os.environ.get("OT_ACT", "1"))          # ot copies on Act
QK_ACT = int(os.environ.get("QK_ACT", "1"))          # qh/kh copies on Act
FIFO_TH = int(os.environ.get("FIFO_TH", "4"))        # attn@v fifo depth
WT_BUFS = int(os.environ.get("WT_BUFS", "10"))       # wt tile buffers
ES_BUFS = int(os.environ.get("ES_BUFS", "4"))        # es tile buffers
PW_POOL = int(os.environ.get("PW_POOL", "0"))        # dedicated pw PSUM pool
BSPLIT = int(os.environ.get("BSPLIT", "1"))          # batch-pair sweep (2 po live)
TAIL_STEPS = int(os.environ.get("TAIL_STEPS", "1"))  # fine-grained tail emission
PREFETCH = int(os.environ.get("PREFETCH", "1"))      # prefetch next-ib eb
QKV_SPLIT = int(os.environ.get("QKV_SPLIT", "1"))    # split first qkv DMA


def tile_paths():
    """Assign each of the 32 (ib, jg) tiles a path, evenly interleaved."""
    counts = {"S": N_S_TILES, "Y": N_Y_TILES, "G": N_G_TILES}
    counts["A"] = 32 - N_S_TILES - N_Y_TILES - N_G_TILES
    err = {k: 0.0 for k in counts}
    seq = []
    for _ in range(32):
        for k in counts:
            err[k] += counts[k] / 32.0
        pick = max(err, key=lambda k: err[k])
        err[pick] -= 1.0
        seq.append(pick)
    return seq


def build_kernel(nc, qkvT, wq, wk, wv, wo, identT, eb, out, reps=1):
    with tile.TileContext(nc) as tc:
        if reps == 1:
            _emit_v2(nc, tc, qkvT, wq, wk, wv, wo, identT, eb, out)
        else:
            with tc.For_i(0, reps, 1):
                _emit_v2(nc, tc, qkvT, wq, wk, wv, wo, identT, eb, out)


def _emit_v2(nc, tc, qkvT, wq, wk, wv, wo, identT, eb, out):
    paths = tile_paths()
    with ExitStack() as ctx:
        consts = ctx.enter_context(tc.tile_pool(name="consts", bufs=1))
        persist = ctx.enter_context(tc.tile_pool(name="persist", bufs=1))
        qkv_pool = ctx.enter_context(tc.tile_pool(name="qkv", bufs=4))
        ebp = ctx.enter_context(tc.tile_pool(name="ebp", bufs=4))
        work = ctx.enter_context(tc.tile_pool(name="work", bufs=8))
        outp = ctx.enter_context(tc.tile_pool(name="outp", bufs=4))
        ps_bufs = (2 if PW_POOL else 3) if BSPLIT else 2
        po_bufs = 2 if BSPLIT else 4
        psS = ctx.enter_context(tc.tile_pool(name="psS", bufs=ps_bufs, space="PSUM"))
        psO = ctx.enter_context(tc.tile_pool(name="psO", bufs=po_bufs, space="PSUM"))
        psW = (ctx.enter_context(tc.tile_pool(name="psW", bufs=2, space="PSUM"))
               if PW_POOL and BSPLIT else psS)

        wq_sb = consts.tile([128, 2, 4 * DH], BF16)
        nc.sync.dma_start(wq_sb[:], wq[:, :, :])
        wk_sb = consts.tile([128, 2, 4 * DH], BF16)
        nc.sync.dma_start(wk_sb[:], wk[:, :, :])
        wv_sb = consts.tile([128, 2, DH], BF16)
        nc.sync.dma_start(wv_sb[:], wv[:, :, :])
        # wo_sb row DH / col D implement the z passthrough: pw[:, D] = z_i
        wo_sb = consts.tile([DH + 1, D + 1], BF16)
        nc.sync.dma_start(wo_sb[:], wo[:, :])
        ident = consts.tile([128, 128], BF16)
        nc.sync.dma_start(ident[:], identT[:, :])

        qh = [persist.tile([128, N], BF16, name=f"qh{b}") for b in range(B)]
        kh = [persist.tile([128, N], BF16, name=f"kh{b}") for b in range(B)]
        vh = [persist.tile([128, NJC, DH + 1], BF16, name=f"vh{b}") for b in range(B)]

        for b in range(B):
            nc.vector.memset(vh[b][:, :, DH : DH + 1], 1.0)

        # ---- projections (one [*, ib] chunk of all batches) ----
        def copy_op(on_act, dst, src):
            if on_act:
                nc.scalar.copy(dst, src)
            else:
                nc.vector.tensor_copy(dst, src)

        def emit_proj_chunk(ib):
            isl = slice(ib * IB, (ib + 1) * IB)
            for b in range(B):
                qkvt = qkv_pool.tile([128, 3, 2, IB], BF16, name="qkvt")
                if ib == 0 and QKV_SPLIT:
                    # split so the q slice (and the first matmuls) land sooner
                    for i3 in range(3):
                        nc.sync.dma_start(qkvt[:, i3], qkvT[b, ib, :, i3])
                else:
                    nc.sync.dma_start(qkvt[:], qkvT[b, ib])
                psq = psS.tile([128, IB], F32, name="psq", tag="s")
                nc.tensor.matmul(psq[:], wq_sb[:, 0, :], qkvt[:, 0, 0, :], start=True, stop=False)
                nc.tensor.matmul(psq[:], wq_sb[:, 1, :], qkvt[:, 0, 1, :], start=False, stop=True)
                copy_op(QK_ACT, qh[b][:, isl], psq[:])

                psk = psS.tile([128, IB], F32, name="psk", tag="s")
                nc.tensor.matmul(psk[:], wk_sb[:, 0, :], qkvt[:, 1, 0, :], start=True, stop=False)
                nc.tensor.matmul(psk[:], wk_sb[:, 1, :], qkvt[:, 1, 1, :], start=False, stop=True)
                copy_op(QK_ACT, kh[b][:, isl], psk[:])

                psv = psS.tile([128, IB // JC, DH], F32, name="psv", tag="s")
                for jl in range(IB // JC):
                    jsl = slice(jl * JC, (jl + 1) * JC)
                    nc.tensor.matmul(psv[:, jl, :], qkvt[:, 2, 0, jsl], wv_sb[:, 0, :],
                                     start=True, stop=False, skip_group_check=True)
                    nc.tensor.matmul(psv[:, jl, :], qkvt[:, 2, 1, jsl], wv_sb[:, 1, :],
                                     start=False, stop=True, skip_group_check=True)
                nc.scalar.copy(vh[b][:, ib * (IB // JC) : (ib + 1) * (IB // JC), 0:DH], psv[:])

        # ---- attention ----
        state = {"u": 0, "fifo": [], "tails": []}

        def pop_attnv():
            ib, po_b, b, jg, wt_ap = state["fifo"].pop(0)
            for t in range(JP):
                jc = jg * JP + t
                nc.tensor.matmul(
                    po_b[:], vh[b][:, jc, :], wt_ap[:, t, :],
                    start=(jc == 0), stop=(jc == NJC - 1),
                )
            if jg == NJG - 1:
                queue_tail(ib, b, po_b)

        def emit_unit(ib, jg, b, po_b, ebt_half, jgl, path):
            isl = slice(ib * IB, (ib + 1) * IB)
            par = state["u"] % 2
            state["u"] += 1
            ps = psS.tile([128, JP, IB], F32, name="ps", tag="s")
            for t in range(JP):
                jc = jg * JP + t
                r0 = 64 * par + 32 * t
                nc.tensor.matmul(
                    ps[:, t, :],
                    kh[b][r0 : r0 + 32, jc * JC : (jc + 1) * JC],
                    qh[b][r0 : r0 + 32, isl],
                    start=True, stop=(path != "Y"),
                    tile_position=(r0, 0),
                )
            if path == "Y":
                for t in range(JP):
                    nc.tensor.matmul(
                        ps[:, t, :], ident[:, :], ebt_half[:, jgl, t, :],
                        start=False, stop=True,
                    )
            if path == "S":
                wti = work.tile([128, JP, IB], I16, name="wt", tag="wt", bufs=WT_BUFS)
                nc.vector.tensor_tensor(
                    wti[:], ps[:], ebt_half[:, jgl].bitcast(I16), ALU.add
                )
                wt_ap = wti[:].bitcast(BF16)
            elif path in ("A", "G"):
                es = work.tile([128, JP, IB], BF16, name="es", tag="es", bufs=ES_BUFS)
                nc.scalar.activation(es[:], ps[:], AF.Exp, scale=1.0 / S_LOG2)
                wtt = work.tile([128, JP, IB], BF16, name="wt", tag="wt", bufs=WT_BUFS)
                meng = nc.gpsimd if path == "G" else nc.vector
                meng.tensor_mul(wtt[:], es[:], ebt_half[:, jgl])
                wt_ap = wtt[:]
            else:  # Y
                wtt = work.tile([128, JP, IB], BF16, name="wt", tag="wt", bufs=WT_BUFS)
                nc.scalar.activation(wtt[:], ps[:], AF.Exp, scale=1.0 / S_LOG2)
                wt_ap = wtt[:]
            state["fifo"].append((ib, po_b, b, jg, wt_ap))
            if state["u"] % 2 == 0:
                while len(state["fifo"]) > FIFO_TH:
                    pop_attnv()
                    pop_attnv()
            if state["tails"]:
                state["tails"].pop(0)()

        def queue_tail(ib, b, po_b):
            # z rides ot row DH (bf16); wo_sb row DH routes it to pw[:, D],
            # transposing it onto the partition axis for the ft scale.
            ot = outp.tile([DH + 1, IB], BF16, name="ot")
            ftb = outp.tile([128, IB // JC, D], F32, name="ftb")

            def step0():
                copy_op(OT_ACT, ot[:], po_b[:])

            def step_s(s):
                def go():
                    pw = psW.tile([128, D + 1], F32, name="pw",
                                  tag=("pw" if PW_POOL else "s"))
                    nc.tensor.matmul(pw[:], ot[:, s * JC : (s + 1) * JC], wo_sb[:],
                                     start=True, stop=True)
                    on_act = FT_ACT == 1 or (FT_ACT == 2 and s % 2 == 0)
                    rts = outp.tile([128, 1], F32, name="rts")
                    nc.vector.reciprocal(rts[:], pw[:, D : D + 1])
                    if on_act:
                        nc.scalar.mul(ftb[:, s, :], pw[:, 0:D], rts[:])
                    else:
                        nc.vector.tensor_scalar_mul(ftb[:, s, :], pw[:, 0:D], rts[:])
                    if s == IB // JC - 1:
                        nc.scalar.dma_start(out[b, ib], ftb[:])
                return go

            if TAIL_STEPS:
                state["tails"].append(step0)
                for s in range(IB // JC):
                    state["tails"].append(step_s(s))
            else:
                def whole():
                    step0()
                    for s in range(IB // JC):
                        step_s(s)()
                state["tails"].append(whole)

        # ---- main sweep ----
        # batch pairs: only 2 po accumulators live at a time, freeing PSUM
        # banks for a third sim buffer (deeper PE/Act/DVE decoupling)
        NJG2 = NJG // 2

        def fetch_eb(ib, half):
            t = ebp.tile([128, NJG2, JP, IB], BF16, name="ebt")
            nc.sync.dma_start(t[:], eb[ib, half])
            return t

        ebts_next = None
        bp_range = range(2) if BSPLIT else range(1)
        bs_of = (lambda bp: (2 * bp, 2 * bp + 1)) if BSPLIT else (lambda bp: tuple(range(B)))
        for ib in range(NIB):
            ebts = ebts_next or [None, None]
            ebts_next = [None, None]
            for bp in bp_range:
                po = {b: psO.tile([DH + 1, IB], F32, name=f"po{b}", tag="po")
                      for b in bs_of(bp)}
                for half in range(2):
                    if ib == 0 and bp == 0 and half == 0:
                        # first projection chunk's data must land before eb
                        emit_proj_chunk(0)
                    if ebts[half] is None:
                        ebts[half] = fetch_eb(ib, half)
                    if PREFETCH and bp == bp_range[-1] and ib + 1 < NIB:
                        # prefetch next i-block's eb during the last pass
                        ebts_next[half] = ebts_next[half] or fetch_eb(ib + 1, half)
                    for jgl in range(NJG2):
                        jg = half * NJG2 + jgl
                        path = paths[ib * NJG + jg]
                        if ib == 0 and bp == 0 and jg % 2 == 0 and jg > 0:
                            emit_proj_chunk(jg // 2)
                        for b in bs_of(bp):
                            emit_unit(ib, jg, b, po[b], ebts[half], jgl, path)
        while state["fifo"]:
            pop_attnv()
        while state["tails"]:
            state["tails"].pop(0)()


_CACHE = {}


def _get_nc(reps=1, var="v2"):
    key = ("nc", reps, var)
    if key not in _CACHE:
        nc = bacc.Bacc("TRN2", target_bir_lowering=False, debug=False, num_devices=NCORES)
        qkvT = nc.dram_tensor("qkvT", [B, NIB, 128, 3, 2, IB], BF16, kind="ExternalInput")
        wq = nc.dram_tensor("wq", [128, 2, 4 * DH], BF16, kind="ExternalInput")
        wk = nc.dram_tensor("wk", [128, 2, 4 * DH], BF16, kind="ExternalInput")
        wv = nc.dram_tensor("wv", [128, 2, DH], BF16, kind="ExternalInput")
        wo = nc.dram_tensor("wo", [DH + 1, D + 1], BF16, kind="ExternalInput")
        identT = nc.dram_tensor("identT", [128, 128], BF16, kind="ExternalInput")
        eb = nc.dram_tensor("eb", [NIB, 2, 128, NJG // 2, JP, IB], BF16, kind="ExternalInput")
        out = nc.dram_tensor("out", [B, NIB, 128, IB // JC, D], F32, kind="ExternalOutput")
        build_kernel(
            nc,
            qkvT.ap(),
            wq.ap(), wk.ap(), wv.ap(), wo.ap(),
            identT.ap(), eb.ap(), out.ap(),
            reps=reps,
        )
        nc.compile()
        _CACHE[key] = nc
    return _CACHE[key]


def _dn_layout(x):
    """[B, N, D] -> [B, NIB, 128, 2, IB]; tile (b, ib)[p, c, col] = x[b, ib*IB+col, c*128+p]."""
    t = x.reshape(B, NIB, IB, 2, 128)
    return np.ascontiguousarray(t.transpose(0, 1, 4, 3, 2).astype(ml_dtypes.bfloat16))


def _w_layout(w, rep):
    """[32, 256] (out, in) -> [128, 2, rep*32] transposed, M-replicated."""
    wt = np.ascontiguousarray(w.T)                       # [256, 32]
    wt = np.concatenate([wt] * rep, axis=1)              # [256, rep*32]
    return np.ascontiguousarray(
        wt.reshape(2, 128, rep * DH).transpose(1, 0, 2).astype(ml_dtypes.bfloat16)
    )


def _eb_layout(pb_h, paths):
    """[N, N] pos_bias head -> [NIB, 2, 128, NJG/2, JP, IB] per-path-coded
    tiles; (ib, half)[p, jgl, t, col] encodes
    pb_h[ib*IB+col, (half*4+jgl)*(JP*128)+t*128+p]."""
    x = pb_h.reshape(NIB, IB, NJG, JP, 128).transpose(0, 2, 4, 3, 1)
    x = np.ascontiguousarray(x)                          # [NIB, NJG, 128, JP, IB] f32
    outb = np.empty(x.shape, dtype=np.uint16)
    for ib in range(NIB):
        for jg in range(NJG):
            p = paths[ib * NJG + jg]
            blk = x[ib, jg]
            if p in ("A", "G"):
                v = np.exp(blk).astype(ml_dtypes.bfloat16)
                outb[ib, jg] = v.view(np.uint16)
            elif p == "Y":
                v = (S_LOG2 * blk).astype(ml_dtypes.bfloat16)
                outb[ib, jg] = v.view(np.uint16)
            else:  # S
                v = np.rint(S_LOG2 * blk + C_SCH).astype(np.int16)
                outb[ib, jg] = v.view(np.uint16)
    # [NIB, NJG, 128, JP, IB] -> [NIB, 2, 128, NJG/2, JP, IB]
    outb = outb.reshape(NIB, 2, NJG // 2, 128, JP, IB).transpose(0, 1, 3, 2, 4, 5)
    return np.ascontiguousarray(outb).view(ml_dtypes.bfloat16)


def make_in_maps(q, k, v, pos_bias, Wq, Wk, Wv, Wo):
    q = np.asarray(q, dtype=np.float32)
    k = np.asarray(k, dtype=np.float32)
    v = np.asarray(v, dtype=np.float32)
    pos_bias = np.asarray(pos_bias, dtype=np.float32)
    Wq = np.asarray(Wq, dtype=np.float32)
    Wk = np.asarray(Wk, dtype=np.float32)
    Wv = np.asarray(Wv, dtype=np.float32)
    Wo = np.asarray(Wo, dtype=np.float32)

    paths = tile_paths()
    qkvT = np.ascontiguousarray(
        np.stack([_dn_layout(q), _dn_layout(k), _dn_layout(v)], axis=3)
    )
    ident = np.eye(128, dtype=np.float32).astype(ml_dtypes.bfloat16)

    in_maps = []
    for h in range(NCORES):
        hs = slice(h * DH, (h + 1) * DH)
        woe = np.zeros((DH + 1, D + 1), dtype=np.float32)
        woe[0:DH, 0:D] = Wo[:, hs].T
        woe[DH, D] = 1.0
        in_maps.append({
            "qkvT": qkvT,
            "wq": _w_layout(SCALE * S_LOG2 * Wq[hs, :], 4),
            "wk": _w_layout(Wk[hs, :], 4),
            "wv": _w_layout(Wv[hs, :], 1),
            "wo": np.ascontiguousarray(woe.astype(ml_dtypes.bfloat16)),
            "identT": ident,
            "eb": _eb_layout(pos_bias[h], paths),
        })
    return in_maps


def kernel(q, k, v, pos_bias, Wq, Wk, Wv, Wo):
    nc = _get_nc()
    in_maps = make_in_maps(q, k, v, pos_bias, Wq, Wk, Wv, Wo)
    res = run_bass_kernel_spmd(nc, in_maps, core_ids=list(range(NCORES)))
    acc = None
    for c in range(NCORES):
        o = res.results[c]["out"].astype(np.float32)
        acc = o if acc is None else acc + o
    # [B, NIB, 128p, 4s, 256] -> [B, N, D] with row i = ib*IB + s*JC + p
    return np.ascontiguousarray(
        acc.transpose(0, 1, 3, 2, 4).reshape(B, N, D)
    )


# revision 48
# speedup vs baseline: 1.4606x; 1.4606x over previous
"""Trainium2 Bass kernel for nn_Attention_2 (8-head attention with positional bias).

Sharding: one head per NeuronCore (8 heads / 8 cores), data-parallel over the
full batch within each core.  Each core computes its head's projections,
attention (unnormalized softmax), and its partial contribution to the output
projection.  The host sums the 8 partial outputs.

v2 design notes:
- qh carries pre-scaled logits x' = (128/ln2) * scale * (q.Wq); kh plain.
- Per (ib, jg) tile, one of three elementwise paths turns sim PSUM into
  attention weights wt (bf16):
    'A': Act exp(ps/S) -> es, DVE mul es*exp(bias)[bf16]      (Act+DVE)
    'Y': PE identity-matmul adds S*bias into PSUM, Act exp(ps/S)  (PE+Act)
    'S': DVE (ps + ebS[i16]) -> int16, bitcast bf16 == Schraudolph exp
         with the bias and all constants folded into ebS        (DVE only)
  The tile counts balance the three engines; 'S' exploits bf16's bit layout
  (value ~= 2^((bits-16256)/128)) so one DVE op does exp+bias-mul.
- Sim matmuls run in 32-row PE groups; consecutive units alternate between
  row pairs (0-63 / 64-127) and the attn@v fifo pops in pairs so two units'
  sim matmuls are adjacent in program order and run concurrently.
- Out-projection in bf16 (4x fewer PE cycles than fp32); z stays f32 in PSUM
  until the single bf16 ot copy.
- DMAs batched: one qkv load per (b, ib), eb per half-ib sweep, one output
  store per (ib, b) into a p-major DRAM layout the host un-permutes.
"""

import os
import sys

sys.path.insert(0, "/opt/trn_rl_repo")

import numpy as np
import ml_dtypes
from contextlib import ExitStack

import concourse.bass as bass  # noqa: F401
import concourse.tile as tile
from concourse import bacc, mybir
from concourse.bass_utils import run_bass_kernel_spmd

B, N, D, H, DH = 4, 2048, 256, 8, 32
SCALE = DH ** -0.5
NCORES = 8
IB = 512            # i-block (query columns per matmul)
NIB = N // IB       # 4
JC = 128            # j-chunk (key rows per partition tile)
NJC = N // JC       # 16
JP = 2              # j-chunks packed per PE pass (row groups)
NJG = NJC // JP     # 8
F32 = mybir.dt.float32
BF16 = mybir.dt.bfloat16
I16 = mybir.dt.int16
AF = mybir.ActivationFunctionType
ALU = mybir.AluOpType

S_LOG2 = 128.0 / np.log(2.0)          # folds exp into bf16 bit layout
C_SCH = 16256.0 - 5.509 + float(os.environ.get("C_SCH_OFF", "0"))
N_S_TILES = int(os.environ.get("N_S_TILES", "18"))   # DVE Schraudolph tiles
N_Y_TILES = int(os.environ.get("N_Y_TILES", "8"))    # PE bias-add tiles
N_G_TILES = int(os.environ.get("N_G_TILES", "0"))    # Act exp + gpsimd mul tiles
FT_ACT = int(os.environ.get("FT_ACT", "1"))          # ft muls on Act engine
OT_ACT = int(os.environ.get("OT_ACT", "1"))          # ot copies on Act
QK_ACT = int(os.environ.get("QK_ACT", "1"))          # qh/kh copies on Act
FIFO_TH = int(os.environ.get("FIFO_TH", "8"))        # attn@v fifo depth
WT_BUFS = int(os.environ.get("WT_BUFS", "18"))       # wt tile buffers
ES_BUFS = int(os.environ.get("ES_BUFS", "4"))        # es tile buffers
PW_POOL = int(os.environ.get("PW_POOL", "0"))        # dedicated pw PSUM pool
BSPLIT = int(os.environ.get("BSPLIT", "1"))          # batch-pair sweep (2 po live)
TAIL_STEPS = int(os.environ.get("TAIL_STEPS", "1"))  # fine-grained tail emission
PREFETCH = int(os.environ.get("PREFETCH", "1"))      # prefetch next-ib eb
QKV_SPLIT = int(os.environ.get("QKV_SPLIT", "1"))    # split first qkv DMA


def tile_paths():
    """Assign each of the 32 (ib, jg) tiles a path, evenly interleaved."""
    counts = {"S": N_S_TILES, "Y": N_Y_TILES, "G": N_G_TILES}
    counts["A"] = 32 - N_S_TILES - N_Y_TILES - N_G_TILES
    err = {k: 0.0 for k in counts}
    seq = []
    for _ in range(32):
        for k in counts:
            err[k] += counts[k] / 32.0
        pick = max(err, key=lambda k: err[k])
        err[pick] -= 1.0
        seq.append(pick)
    return seq


def build_kernel(nc, qkvT, wq, wk, wv, wo, identT, eb, out, reps=1):
    with tile.TileContext(nc) as tc:
        if reps == 1:
            _emit_v2(nc, tc, qkvT, wq, wk, wv, wo, identT, eb, out)
        else:
            with tc.For_i(0, reps, 1):
                _emit_v2(nc, tc, qkvT, wq, wk, wv, wo, identT, eb, out)


def _emit_v2(nc, tc, qkvT, wq, wk, wv, wo, identT, eb, out):
    paths = tile_paths()
    with ExitStack() as ctx:
        consts = ctx.enter_context(tc.tile_pool(name="consts", bufs=1))
        persist = ctx.enter_context(tc.tile_pool(name="persist", bufs=1))
        qkv_pool = ctx.enter_context(tc.tile_pool(name="qkv", bufs=4))
        ebp = ctx.enter_context(tc.tile_pool(name="ebp", bufs=4))
        work = ctx.enter_context(tc.tile_pool(name="work", bufs=8))
        outp = ctx.enter_context(tc.tile_pool(name="outp", bufs=4))
        ps_bufs = (2 if PW_POOL else 3) if BSPLIT else 2
        po_bufs = 2 if BSPLIT else 4
        psS = ctx.enter_context(tc.tile_pool(name="psS", bufs=ps_bufs, space="PSUM"))
        psO = ctx.enter_context(tc.tile_pool(name="psO", bufs=po_bufs, space="PSUM"))
        psW = (ctx.enter_context(tc.tile_pool(name="psW", bufs=2, space="PSUM"))
               if PW_POOL and BSPLIT else psS)

        wq_sb = consts.tile([128, 2, 4 * DH], BF16)
        nc.sync.dma_start(wq_sb[:], wq[:, :, :])
        wk_sb = consts.tile([128, 2, 4 * DH], BF16)
        nc.sync.dma_start(wk_sb[:], wk[:, :, :])
        wv_sb = consts.tile([128, 2, DH], BF16)
        nc.sync.dma_start(wv_sb[:], wv[:, :, :])
        # wo_sb row DH / col D implement the z passthrough: pw[:, D] = z_i
        wo_sb = consts.tile([DH + 1, D + 1], BF16)
        nc.sync.dma_start(wo_sb[:], wo[:, :])
        ident = consts.tile([128, 128], BF16)
        nc.sync.dma_start(ident[:], identT[:, :])

        qh = [persist.tile([128, N], BF16, name=f"qh{b}") for b in range(B)]
        kh = [persist.tile([128, N], BF16, name=f"kh{b}") for b in range(B)]
        vh = [persist.tile([128, NJC, DH + 1], BF16, name=f"vh{b}") for b in range(B)]

        for b in range(B):
            nc.vector.memset(vh[b][:, :, DH : DH + 1], 1.0)

        # ---- projections (one [*, ib] chunk of all batches) ----
        def copy_op(on_act, dst, src):
            if on_act:
                nc.scalar.copy(dst, src)
            else:
                nc.vector.tensor_copy(dst, src)

        def emit_proj_chunk(ib):
            isl = slice(ib * IB, (ib + 1) * IB)
            for b in range(B):
                qkvt = qkv_pool.tile([128, 3, 2, IB], BF16, name="qkvt")
                if ib == 0 and QKV_SPLIT:
                    # split so the q slice (and the first matmuls) land sooner
                    for i3 in range(3):
                        nc.sync.dma_start(qkvt[:, i3], qkvT[b, ib, :, i3])
                else:
                    nc.sync.dma_start(qkvt[:], qkvT[b, ib])
                psq = psS.tile([128, IB], F32, name="psq", tag="s")
                nc.tensor.matmul(psq[:], wq_sb[:, 0, :], qkvt[:, 0, 0, :], start=True, stop=False)
                nc.tensor.matmul(psq[:], wq_sb[:, 1, :], qkvt[:, 0, 1, :], start=False, stop=True)
                copy_op(QK_ACT, qh[b][:, isl], psq[:])

                psk = psS.tile([128, IB], F32, name="psk", tag="s")
                nc.tensor.matmul(psk[:], wk_sb[:, 0, :], qkvt[:, 1, 0, :], start=True, stop=False)
                nc.tensor.matmul(psk[:], wk_sb[:, 1, :], qkvt[:, 1, 1, :], start=False, stop=True)
                copy_op(QK_ACT, kh[b][:, isl], psk[:])

                psv = psS.tile([128, IB // JC, DH], F32, name="psv", tag="s")
                for jl in range(IB // JC):
                    jsl = slice(jl * JC, (jl + 1) * JC)
                    nc.tensor.matmul(psv[:, jl, :], qkvt[:, 2, 0, jsl], wv_sb[:, 0, :],
                                     start=True, stop=False, skip_group_check=True)
                    nc.tensor.matmul(psv[:, jl, :], qkvt[:, 2, 1, jsl], wv_sb[:, 1, :],
                                     start=False, stop=True, skip_group_check=True)
                nc.scalar.copy(vh[b][:, ib * (IB // JC) : (ib + 1) * (IB // JC), 0:DH], psv[:])

        # ---- attention ----
        state = {"u": 0, "fifo": [], "tails": []}

        def pop_attnv():
            ib, po_b, b, jg, wt_ap = state["fifo"].pop(0)
            for t in range(JP):
                jc = jg * JP + t
                nc.tensor.matmul(
                    po_b[:], vh[b][:, jc, :], wt_ap[:, t, :],
                    start=(jc == 0), stop=(jc == NJC - 1),
                )
            if jg == NJG - 1:
                queue_tail(ib, b, po_b)

        def emit_unit(ib, jg, b, po_b, ebt_half, jgl, path):
            isl = slice(ib * IB, (ib + 1) * IB)
            par = state["u"] % 2
            state["u"] += 1
            ps = psS.tile([128, JP, IB], F32, name="ps", tag="s")
            for t in range(JP):
                jc = jg * JP + t
                r0 = 64 * par + 32 * t
                nc.tensor.matmul(
                    ps[:, t, :],
                    kh[b][r0 : r0 + 32, jc * JC : (jc + 1) * JC],
                    qh[b][r0 : r0 + 32, isl],
                    start=True, stop=(path != "Y"),
                    tile_position=(r0, 0),
                )
            if path == "Y":
                for t in range(JP):
                    nc.tensor.matmul(
                        ps[:, t, :], ident[:, :], ebt_half[:, jgl, t, :],
                        start=False, stop=True,
                    )
            if path == "S":
                wti = work.tile([128, JP, IB], I16, name="wt", tag="wt", bufs=WT_BUFS)
                nc.vector.tensor_tensor(
                    wti[:], ps[:], ebt_half[:, jgl].bitcast(I16), ALU.add
                )
                wt_ap = wti[:].bitcast(BF16)
            elif path in ("A", "G"):
                es = work.tile([128, JP, IB], BF16, name="es", tag="es", bufs=ES_BUFS)
                nc.scalar.activation(es[:], ps[:], AF.Exp, scale=1.0 / S_LOG2)
                wtt = work.tile([128, JP, IB], BF16, name="wt", tag="wt", bufs=WT_BUFS)
                meng = nc.gpsimd if path == "G" else nc.vector
                meng.tensor_mul(wtt[:], es[:], ebt_half[:, jgl])
                wt_ap = wtt[:]
            else:  # Y
                wtt = work.tile([128, JP, IB], BF16, name="wt", tag="wt", bufs=WT_BUFS)
                nc.scalar.activation(wtt[:], ps[:], AF.Exp, scale=1.0 / S_LOG2)
                wt_ap = wtt[:]
            state["fifo"].append((ib, po_b, b, jg, wt_ap))
            if state["u"] % 2 == 0:
                while len(state["fifo"]) > FIFO_TH:
                    pop_attnv()
                    pop_attnv()
            if state["tails"]:
                state["tails"].pop(0)()

        def queue_tail(ib, b, po_b):
            # z rides ot row DH (bf16); wo_sb row DH routes it to pw[:, D],
            # transposing it onto the partition axis for the ft scale.
            ot = outp.tile([DH + 1, IB], BF16, name="ot")
            ftb = outp.tile([128, IB // JC, D], F32, name="ftb")

            def step0():
                copy_op(OT_ACT, ot[:], po_b[:])

            def step_s(s):
                def go():
                    pw = psW.tile([128, D + 1], F32, name="pw",
                                  tag=("pw" if PW_POOL else "s"))
                    nc.tensor.matmul(pw[:], ot[:, s * JC : (s + 1) * JC], wo_sb[:],
                                     start=True, stop=True)
                    on_act = FT_ACT == 1 or (FT_ACT == 2 and s % 2 == 0)
                    rts = outp.tile([128, 1], F32, name="rts")
                    nc.vector.reciprocal(rts[:], pw[:, D : D + 1])
                    if on_act:
                        nc.scalar.mul(ftb[:, s, :], pw[:, 0:D], rts[:])
                    else:
                        nc.vector.tensor_scalar_mul(ftb[:, s, :], pw[:, 0:D], rts[:])
                    if s == IB // JC - 1:
                        nc.scalar.dma_start(out[b, ib], ftb[:])
                return go

            if TAIL_STEPS:
                state["tails"].append(step0)
                for s in range(IB // JC):
                    state["tails"].append(step_s(s))
            else:
                def whole():
                    step0()
                    for s in range(IB // JC):
                        step_s(s)()
                state["tails"].append(whole)

        # ---- main sweep ----
        # batch pairs: only 2 po accumulators live at a time, freeing PSUM
        # banks for a third sim buffer (deeper PE/Act/DVE decoupling)
        NJG2 = NJG // 2

        def fetch_eb(ib, half):
            t = ebp.tile([128, NJG2, JP, IB], BF16, name="ebt")
            nc.sync.dma_start(t[:], eb[ib, half])
            return t

        ebts_next = None
        bp_range = range(2) if BSPLIT else range(1)
        bs_of = (lambda bp: (2 * bp, 2 * bp + 1)) if BSPLIT else (lambda bp: tuple(range(B)))
        for ib in range(NIB):
            ebts = ebts_next or [None, None]
            ebts_next = [None, None]
            for bp in bp_range:
                po = {b: psO.tile([DH + 1, IB], F32, name=f"po{b}", tag="po")
                      for b in bs_of(bp)}
                for half in range(2):
                    if ib == 0 and bp == 0 and half == 0:
                        # first projection chunk's data must land before eb
                        emit_proj_chunk(0)
                    if ebts[half] is None:
                        ebts[half] = fetch_eb(ib, half)
                    if PREFETCH and bp == bp_range[-1] and ib + 1 < NIB:
                        # prefetch next i-block's eb during the last pass
                        ebts_next[half] = ebts_next[half] or fetch_eb(ib + 1, half)
                    for jgl in range(NJG2):
                        jg = half * NJG2 + jgl
                        path = paths[ib * NJG + jg]
                        if ib == 0 and bp == 0 and jg % 2 == 0 and jg > 0:
                            emit_proj_chunk(jg // 2)
                        for b in bs_of(bp):
                            emit_unit(ib, jg, b, po[b], ebts[half], jgl, path)
        while state["fifo"]:
            pop_attnv()
        while state["tails"]:
            state["tails"].pop(0)()


_CACHE = {}


def _get_nc(reps=1, var="v2"):
    key = ("nc", reps, var)
    if key not in _CACHE:
        nc = bacc.Bacc("TRN2", target_bir_lowering=False, debug=False, num_devices=NCORES)
        qkvT = nc.dram_tensor("qkvT", [B, NIB, 128, 3, 2, IB], BF16, kind="ExternalInput")
        wq = nc.dram_tensor("wq", [128, 2, 4 * DH], BF16, kind="ExternalInput")
        wk = nc.dram_tensor("wk", [128, 2, 4 * DH], BF16, kind="ExternalInput")
        wv = nc.dram_tensor("wv", [128, 2, DH], BF16, kind="ExternalInput")
        wo = nc.dram_tensor("wo", [DH + 1, D + 1], BF16, kind="ExternalInput")
        identT = nc.dram_tensor("identT", [128, 128], BF16, kind="ExternalInput")
        eb = nc.dram_tensor("eb", [NIB, 2, 128, NJG // 2, JP, IB], BF16, kind="ExternalInput")
        out = nc.dram_tensor("out", [B, NIB, 128, IB // JC, D], F32, kind="ExternalOutput")
        build_kernel(
            nc,
            qkvT.ap(),
            wq.ap(), wk.ap(), wv.ap(), wo.ap(),
            identT.ap(), eb.ap(), out.ap(),
            reps=reps,
        )
        nc.compile()
        _CACHE[key] = nc
    return _CACHE[key]


def _dn_layout(x):
    """[B, N, D] -> [B, NIB, 128, 2, IB]; tile (b, ib)[p, c, col] = x[b, ib*IB+col, c*128+p]."""
    t = x.reshape(B, NIB, IB, 2, 128)
    return np.ascontiguousarray(t.transpose(0, 1, 4, 3, 2).astype(ml_dtypes.bfloat16))


def _w_layout(w, rep):
    """[32, 256] (out, in) -> [128, 2, rep*32] transposed, M-replicated."""
    wt = np.ascontiguousarray(w.T)                       # [256, 32]
    wt = np.concatenate([wt] * rep, axis=1)              # [256, rep*32]
    return np.ascontiguousarray(
        wt.reshape(2, 128, rep * DH).transpose(1, 0, 2).astype(ml_dtypes.bfloat16)
    )


def _eb_layout(pb_h, paths):
    """[N, N] pos_bias head -> [NIB, 2, 128, NJG/2, JP, IB] per-path-coded
    tiles; (ib, half)[p, jgl, t, col] encodes
    pb_h[ib*IB+col, (half*4+jgl)*(JP*128)+t*128+p]."""
    x = pb_h.reshape(NIB, IB, NJG, JP, 128).transpose(0, 2, 4, 3, 1)
    x = np.ascontiguousarray(x)                          # [NIB, NJG, 128, JP, IB] f32
    outb = np.empty(x.shape, dtype=np.uint16)
    for ib in range(NIB):
        for jg in range(NJG):
            p = paths[ib * NJG + jg]
            blk = x[ib, jg]
            if p in ("A", "G"):
                v = np.exp(blk).astype(ml_dtypes.bfloat16)
                outb[ib, jg] = v.view(np.uint16)
            elif p == "Y":
                v = (S_LOG2 * blk).astype(ml_dtypes.bfloat16)
                outb[ib, jg] = v.view(np.uint16)
            else:  # S
                v = np.rint(S_LOG2 * blk + C_SCH).astype(np.int16)
                outb[ib, jg] = v.view(np.uint16)
    # [NIB, NJG, 128, JP, IB] -> [NIB, 2, 128, NJG/2, JP, IB]
    outb = outb.reshape(NIB, 2, NJG // 2, 128, JP, IB).transpose(0, 1, 3, 2, 4, 5)
    return np.ascontiguousarray(outb).view(ml_dtypes.bfloat16)


def make_in_maps(q, k, v, pos_bias, Wq, Wk, Wv, Wo):
    q = np.asarray(q, dtype=np.float32)
    k = np.asarray(k, dtype=np.float32)
    v = np.asarray(v, dtype=np.float32)
    pos_bias = np.asarray(pos_bias, dtype=np.float32)
    Wq = np.asarray(Wq, dtype=np.float32)
    Wk = np.asarray(Wk, dtype=np.float32)
    Wv = np.asarray(Wv, dtype=np.float32)
    Wo = np.asarray(Wo, dtype=np.float32)

    paths = tile_paths()
    qkvT = np.ascontiguousarray(
        np.stack([_dn_layout(q), _dn_layout(k), _dn_layout(v)], axis=3)
    )
    ident = np.eye(128, dtype=np.float32).astype(ml_dtypes.bfloat16)

    in_maps = []
    for h in range(NCORES):
        hs = slice(h * DH, (h + 1) * DH)
        woe = np.zeros((DH + 1, D + 1), dtype=np.float32)
        woe[0:DH, 0:D] = Wo[:, hs].T
        woe[DH, D] = 1.0
        in_maps.append({
            "qkvT": qkvT,
            "wq": _w_layout(SCALE * S_LOG2 * Wq[hs, :], 4),
            "wk": _w_layout(Wk[hs, :], 4),
            "wv": _w_layout(Wv[hs, :], 1),
            "wo": np.ascontiguousarray(woe.astype(ml_dtypes.bfloat16)),
            "identT": ident,
            "eb": _eb_layout(pos_bias[h], paths),
        })
    return in_maps


def kernel(q, k, v, pos_bias, Wq, Wk, Wv, Wo):
    nc = _get_nc()
    in_maps = make_in_maps(q, k, v, pos_bias, Wq, Wk, Wv, Wo)
    res = run_bass_kernel_spmd(nc, in_maps, core_ids=list(range(NCORES)))
    acc = None
    for c in range(NCORES):
        o = res.results[c]["out"].astype(np.float32)
        acc = o if acc is None else acc + o
    # [B, NIB, 128p, 4s, 256] -> [B, N, D] with row i = ib*IB + s*JC + p
    return np.ascontiguousarray(
        acc.transpose(0, 1, 3, 2, 4).reshape(B, N, D)
    )


# revision 52
# speedup vs baseline: 1.5033x; 1.0293x over previous
"""Trainium2 Bass kernel for nn_Attention_2 (8-head attention with positional bias).

Sharding: one head per NeuronCore (8 heads / 8 cores), data-parallel over the
full batch within each core.  Each core computes its head's projections,
attention (unnormalized softmax), and its partial contribution to the output
projection.  The host sums the 8 partial outputs.

v2 design notes:
- qh carries pre-scaled logits x' = (128/ln2) * scale * (q.Wq); kh plain.
- Per (ib, jg) tile, one of three elementwise paths turns sim PSUM into
  attention weights wt (bf16):
    'A': Act exp(ps/S) -> es, DVE mul es*exp(bias)[bf16]      (Act+DVE)
    'Y': PE identity-matmul adds S*bias into PSUM, Act exp(ps/S)  (PE+Act)
    'S': DVE (ps + ebS[i16]) -> int16, bitcast bf16 == Schraudolph exp
         with the bias and all constants folded into ebS        (DVE only)
  The tile counts balance the three engines; 'S' exploits bf16's bit layout
  (value ~= 2^((bits-16256)/128)) so one DVE op does exp+bias-mul.
- Sim matmuls run in 32-row PE groups; consecutive units alternate between
  row pairs (0-63 / 64-127) and the attn@v fifo pops in pairs so two units'
  sim matmuls are adjacent in program order and run concurrently.
- Out-projection in bf16 (4x fewer PE cycles than fp32); z stays f32 in PSUM
  until the single bf16 ot copy.
- DMAs batched: one qkv load per (b, ib), eb per half-ib sweep, one output
  store per (ib, b) into a p-major DRAM layout the host un-permutes.
"""

import os
import sys

sys.path.insert(0, "/opt/trn_rl_repo")

import numpy as np
import ml_dtypes
from contextlib import ExitStack

import concourse.bass as bass  # noqa: F401
import concourse.tile as tile
from concourse import bacc, mybir
from concourse.bass_utils import run_bass_kernel_spmd

B, N, D, H, DH = 4, 2048, 256, 8, 32
SCALE = DH ** -0.5
NCORES = 8
IB = 512            # i-block (query columns per matmul)
NIB = N // IB       # 4
JC = 128            # j-chunk (key rows per partition tile)
NJC = N // JC       # 16
JP = 2              # j-chunks packed per PE pass (row groups)
NJG = NJC // JP     # 8
F32 = mybir.dt.float32
BF16 = mybir.dt.bfloat16
I16 = mybir.dt.int16
AF = mybir.ActivationFunctionType
ALU = mybir.AluOpType

S_LOG2 = 128.0 / np.log(2.0)          # folds exp into bf16 bit layout
C_SCH = 16256.0 - 5.509 + float(os.environ.get("C_SCH_OFF", "0"))
N_S_TILES = int(os.environ.get("N_S_TILES", "18"))   # DVE Schraudolph tiles
N_Y_TILES = int(os.environ.get("N_Y_TILES", "8"))    # PE bias-add tiles
N_G_TILES = int(os.environ.get("N_G_TILES", "0"))    # Act exp + gpsimd mul tiles
FT_ACT = int(os.environ.get("FT_ACT", "1"))          # ft muls on Act engine
OT_ACT = int(os.environ.get("OT_ACT", "1"))          # ot copies on Act
QK_ACT = int(os.environ.get("QK_ACT", "1"))          # qh/kh copies on Act
FIFO_TH = int(os.environ.get("FIFO_TH", "8"))        # attn@v fifo depth
WT_BUFS = int(os.environ.get("WT_BUFS", "18"))       # wt tile buffers
ES_BUFS = int(os.environ.get("ES_BUFS", "4"))        # es tile buffers
PW_POOL = int(os.environ.get("PW_POOL", "0"))        # dedicated pw PSUM pool
BSPLIT = int(os.environ.get("BSPLIT", "1"))          # batch-pair sweep (2 po live)
TAIL_STEPS = int(os.environ.get("TAIL_STEPS", "1"))  # fine-grained tail emission
PREFETCH = int(os.environ.get("PREFETCH", "1"))      # prefetch next-ib eb
QKV_SPLIT = int(os.environ.get("QKV_SPLIT", "1"))    # split first qkv DMA
ENDGAME = int(os.environ.get("ENDGAME", "1"))        # taper fifo/tails at the end


def tile_paths():
    """Assign each of the 32 (ib, jg) tiles a path, evenly interleaved."""
    counts = {"S": N_S_TILES, "Y": N_Y_TILES, "G": N_G_TILES}
    counts["A"] = 32 - N_S_TILES - N_Y_TILES - N_G_TILES
    err = {k: 0.0 for k in counts}
    seq = []
    for _ in range(32):
        for k in counts:
            err[k] += counts[k] / 32.0
        pick = max(err, key=lambda k: err[k])
        err[pick] -= 1.0
        seq.append(pick)
    return seq


def build_kernel(nc, qkvT, wq, wk, wv, wo, identT, eb, out, reps=1):
    with tile.TileContext(nc) as tc:
        if reps == 1:
            _emit_v2(nc, tc, qkvT, wq, wk, wv, wo, identT, eb, out)
        else:
            with tc.For_i(0, reps, 1):
                _emit_v2(nc, tc, qkvT, wq, wk, wv, wo, identT, eb, out)


def _emit_v2(nc, tc, qkvT, wq, wk, wv, wo, identT, eb, out):
    paths = tile_paths()
    with ExitStack() as ctx:
        consts = ctx.enter_context(tc.tile_pool(name="consts", bufs=1))
        persist = ctx.enter_context(tc.tile_pool(name="persist", bufs=1))
        qkv_pool = ctx.enter_context(tc.tile_pool(name="qkv", bufs=4))
        ebp = ctx.enter_context(tc.tile_pool(name="ebp", bufs=4))
        work = ctx.enter_context(tc.tile_pool(name="work", bufs=8))
        outp = ctx.enter_context(tc.tile_pool(name="outp", bufs=4))
        ps_bufs = (2 if PW_POOL else 3) if BSPLIT else 2
        po_bufs = 2 if BSPLIT else 4
        psS = ctx.enter_context(tc.tile_pool(name="psS", bufs=ps_bufs, space="PSUM"))
        psO = ctx.enter_context(tc.tile_pool(name="psO", bufs=po_bufs, space="PSUM"))
        psW = (ctx.enter_context(tc.tile_pool(name="psW", bufs=2, space="PSUM"))
               if PW_POOL and BSPLIT else psS)

        wq_sb = consts.tile([128, 2, 4 * DH], BF16)
        nc.sync.dma_start(wq_sb[:], wq[:, :, :])
        wk_sb = consts.tile([128, 2, 4 * DH], BF16)
        nc.sync.dma_start(wk_sb[:], wk[:, :, :])
        wv_sb = consts.tile([128, 2, DH], BF16)
        nc.sync.dma_start(wv_sb[:], wv[:, :, :])
        # wo_sb row DH / col D implement the z passthrough: pw[:, D] = z_i
        wo_sb = consts.tile([DH + 1, D + 1], BF16)
        nc.sync.dma_start(wo_sb[:], wo[:, :])
        ident = consts.tile([128, 128], BF16)
        nc.sync.dma_start(ident[:], identT[:, :])

        qh = [persist.tile([128, N], BF16, name=f"qh{b}") for b in range(B)]
        kh = [persist.tile([128, N], BF16, name=f"kh{b}") for b in range(B)]
        vh = [persist.tile([128, NJC, DH + 1], BF16, name=f"vh{b}") for b in range(B)]

        for b in range(B):
            nc.vector.memset(vh[b][:, :, DH : DH + 1], 1.0)

        # ---- projections (one [*, ib] chunk of all batches) ----
        def copy_op(on_act, dst, src):
            if on_act:
                nc.scalar.copy(dst, src)
            else:
                nc.vector.tensor_copy(dst, src)

        def emit_proj_chunk(ib):
            isl = slice(ib * IB, (ib + 1) * IB)
            for b in range(B):
                qkvt = qkv_pool.tile([128, 3, 2, IB], BF16, name="qkvt")
                if ib == 0 and QKV_SPLIT:
                    # split so the q slice (and the first matmuls) land sooner
                    for i3 in range(3):
                        nc.sync.dma_start(qkvt[:, i3], qkvT[b, ib, :, i3])
                else:
                    nc.sync.dma_start(qkvt[:], qkvT[b, ib])
                psq = psS.tile([128, IB], F32, name="psq", tag="s")
                nc.tensor.matmul(psq[:], wq_sb[:, 0, :], qkvt[:, 0, 0, :], start=True, stop=False)
                nc.tensor.matmul(psq[:], wq_sb[:, 1, :], qkvt[:, 0, 1, :], start=False, stop=True)
                copy_op(QK_ACT, qh[b][:, isl], psq[:])

                psk = psS.tile([128, IB], F32, name="psk", tag="s")
                nc.tensor.matmul(psk[:], wk_sb[:, 0, :], qkvt[:, 1, 0, :], start=True, stop=False)
                nc.tensor.matmul(psk[:], wk_sb[:, 1, :], qkvt[:, 1, 1, :], start=False, stop=True)
                copy_op(QK_ACT, kh[b][:, isl], psk[:])

                psv = psS.tile([128, IB // JC, DH], F32, name="psv", tag="s")
                for jl in range(IB // JC):
                    jsl = slice(jl * JC, (jl + 1) * JC)
                    nc.tensor.matmul(psv[:, jl, :], qkvt[:, 2, 0, jsl], wv_sb[:, 0, :],
                                     start=True, stop=False, skip_group_check=True)
                    nc.tensor.matmul(psv[:, jl, :], qkvt[:, 2, 1, jsl], wv_sb[:, 1, :],
                                     start=False, stop=True, skip_group_check=True)
                nc.scalar.copy(vh[b][:, ib * (IB // JC) : (ib + 1) * (IB // JC), 0:DH], psv[:])

        # ---- attention ----
        state = {"u": 0, "fifo": [], "tails": []}

        def pop_attnv():
            ib, po_b, b, jg, wt_ap = state["fifo"].pop(0)
            for t in range(JP):
                jc = jg * JP + t
                nc.tensor.matmul(
                    po_b[:], vh[b][:, jc, :], wt_ap[:, t, :],
                    start=(jc == 0), stop=(jc == NJC - 1),
                )
            if jg == NJG - 1:
                queue_tail(ib, b, po_b)

        def emit_unit(ib, jg, b, po_b, ebt_half, jgl, path):
            isl = slice(ib * IB, (ib + 1) * IB)
            par = state["u"] % 2
            state["u"] += 1
            ps = psS.tile([128, JP, IB], F32, name="ps", tag="s")
            for t in range(JP):
                jc = jg * JP + t
                r0 = 64 * par + 32 * t
                nc.tensor.matmul(
                    ps[:, t, :],
                    kh[b][r0 : r0 + 32, jc * JC : (jc + 1) * JC],
                    qh[b][r0 : r0 + 32, isl],
                    start=True, stop=(path != "Y"),
                    tile_position=(r0, 0),
                )
            if path == "Y":
                for t in range(JP):
                    nc.tensor.matmul(
                        ps[:, t, :], ident[:, :], ebt_half[:, jgl, t, :],
                        start=False, stop=True,
                    )
            if path == "S":
                wti = work.tile([128, JP, IB], I16, name="wt", tag="wt", bufs=WT_BUFS)
                nc.vector.tensor_tensor(
                    wti[:], ps[:], ebt_half[:, jgl].bitcast(I16), ALU.add
                )
                wt_ap = wti[:].bitcast(BF16)
            elif path in ("A", "G"):
                es = work.tile([128, JP, IB], BF16, name="es", tag="es", bufs=ES_BUFS)
                nc.scalar.activation(es[:], ps[:], AF.Exp, scale=1.0 / S_LOG2)
                wtt = work.tile([128, JP, IB], BF16, name="wt", tag="wt", bufs=WT_BUFS)
                meng = nc.gpsimd if path == "G" else nc.vector
                meng.tensor_mul(wtt[:], es[:], ebt_half[:, jgl])
                wt_ap = wtt[:]
            else:  # Y
                wtt = work.tile([128, JP, IB], BF16, name="wt", tag="wt", bufs=WT_BUFS)
                nc.scalar.activation(wtt[:], ps[:], AF.Exp, scale=1.0 / S_LOG2)
                wt_ap = wtt[:]
            state["fifo"].append((ib, po_b, b, jg, wt_ap))
            th = 2 if state.get("endgame") else FIFO_TH
            if state["u"] % 2 == 0:
                while len(state["fifo"]) > th:
                    pop_attnv()
                    pop_attnv()
            for _ in range(2 if state.get("endgame") else 1):
                if state["tails"]:
                    state["tails"].pop(0)()

        def queue_tail(ib, b, po_b):
            # z rides ot row DH (bf16); wo_sb row DH routes it to pw[:, D],
            # transposing it onto the partition axis for the ft scale.
            ot = outp.tile([DH + 1, IB], BF16, name="ot")
            ftb = outp.tile([128, IB // JC, D], F32, name="ftb")

            def step0():
                copy_op(OT_ACT, ot[:], po_b[:])

            def step_s(s):
                def go():
                    pw = psW.tile([128, D + 1], F32, name="pw",
                                  tag=("pw" if PW_POOL else "s"))
                    nc.tensor.matmul(pw[:], ot[:, s * JC : (s + 1) * JC], wo_sb[:],
                                     start=True, stop=True)
                    on_act = FT_ACT == 1 or (FT_ACT == 2 and s % 2 == 0)
                    rts = outp.tile([128, 1], F32, name="rts")
                    nc.vector.reciprocal(rts[:], pw[:, D : D + 1])
                    if on_act:
                        nc.scalar.mul(ftb[:, s, :], pw[:, 0:D], rts[:])
                    else:
                        nc.vector.tensor_scalar_mul(ftb[:, s, :], pw[:, 0:D], rts[:])
                    if s == IB // JC - 1:
                        nc.scalar.dma_start(out[b, ib], ftb[:])
                return go

            if TAIL_STEPS:
                state["tails"].append(step0)
                for s in range(IB // JC):
                    state["tails"].append(step_s(s))
            else:
                def whole():
                    step0()
                    for s in range(IB // JC):
                        step_s(s)()
                state["tails"].append(whole)

        # ---- main sweep ----
        # batch pairs: only 2 po accumulators live at a time, freeing PSUM
        # banks for a third sim buffer (deeper PE/Act/DVE decoupling)
        NJG2 = NJG // 2

        def fetch_eb(ib, half):
            t = ebp.tile([128, NJG2, JP, IB], BF16, name="ebt")
            nc.sync.dma_start(t[:], eb[ib, half])
            return t

        ebts_next = None
        bp_range = range(2) if BSPLIT else range(1)
        bs_of = (lambda bp: (2 * bp, 2 * bp + 1)) if BSPLIT else (lambda bp: tuple(range(B)))
        for ib in range(NIB):
            ebts = ebts_next or [None, None]
            ebts_next = [None, None]
            for bp in bp_range:
                po = {b: psO.tile([DH + 1, IB], F32, name=f"po{b}", tag="po")
                      for b in bs_of(bp)}
                for half in range(2):
                    if ib == 0 and bp == 0 and half == 0:
                        # first projection chunk's data must land before eb
                        emit_proj_chunk(0)
                    if ebts[half] is None:
                        ebts[half] = fetch_eb(ib, half)
                    if PREFETCH and bp == bp_range[-1] and ib + 1 < NIB:
                        # prefetch next i-block's eb during the last pass
                        ebts_next[half] = ebts_next[half] or fetch_eb(ib + 1, half)
                    if ENDGAME and ib == NIB - 1 and bp == bp_range[-1] and half == 1:
                        # drain fifo/tails aggressively so the epilogue is short
                        state["endgame"] = True
                    for jgl in range(NJG2):
                        jg = half * NJG2 + jgl
                        path = paths[ib * NJG + jg]
                        if ib == 0 and bp == 0 and jg % 2 == 0 and jg > 0:
                            emit_proj_chunk(jg // 2)
                        for b in bs_of(bp):
                            emit_unit(ib, jg, b, po[b], ebts[half], jgl, path)
        while state["fifo"]:
            pop_attnv()
        while state["tails"]:
            state["tails"].pop(0)()


_CACHE = {}


def _get_nc(reps=1, var="v2"):
    key = ("nc", reps, var)
    if key not in _CACHE:
        nc = bacc.Bacc("TRN2", target_bir_lowering=False, debug=False, num_devices=NCORES)
        qkvT = nc.dram_tensor("qkvT", [B, NIB, 128, 3, 2, IB], BF16, kind="ExternalInput")
        wq = nc.dram_tensor("wq", [128, 2, 4 * DH], BF16, kind="ExternalInput")
        wk = nc.dram_tensor("wk", [128, 2, 4 * DH], BF16, kind="ExternalInput")
        wv = nc.dram_tensor("wv", [128, 2, DH], BF16, kind="ExternalInput")
        wo = nc.dram_tensor("wo", [DH + 1, D + 1], BF16, kind="ExternalInput")
        identT = nc.dram_tensor("identT", [128, 128], BF16, kind="ExternalInput")
        eb = nc.dram_tensor("eb", [NIB, 2, 128, NJG // 2, JP, IB], BF16, kind="ExternalInput")
        out = nc.dram_tensor("out", [B, NIB, 128, IB // JC, D], F32, kind="ExternalOutput")
        build_kernel(
            nc,
            qkvT.ap(),
            wq.ap(), wk.ap(), wv.ap(), wo.ap(),
            identT.ap(), eb.ap(), out.ap(),
            reps=reps,
        )
        nc.compile()
        _CACHE[key] = nc
    return _CACHE[key]


def _dn_layout(x):
    """[B, N, D] -> [B, NIB, 128, 2, IB]; tile (b, ib)[p, c, col] = x[b, ib*IB+col, c*128+p]."""
    t = x.reshape(B, NIB, IB, 2, 128)
    return np.ascontiguousarray(t.transpose(0, 1, 4, 3, 2).astype(ml_dtypes.bfloat16))


def _w_layout(w, rep):
    """[32, 256] (out, in) -> [128, 2, rep*32] transposed, M-replicated."""
    wt = np.ascontiguousarray(w.T)                       # [256, 32]
    wt = np.concatenate([wt] * rep, axis=1)              # [256, rep*32]
    return np.ascontiguousarray(
        wt.reshape(2, 128, rep * DH).transpose(1, 0, 2).astype(ml_dtypes.bfloat16)
    )


def _eb_layout(pb_h, paths):
    """[N, N] pos_bias head -> [NIB, 2, 128, NJG/2, JP, IB] per-path-coded
    tiles; (ib, half)[p, jgl, t, col] encodes
    pb_h[ib*IB+col, (half*4+jgl)*(JP*128)+t*128+p]."""
    x = pb_h.reshape(NIB, IB, NJG, JP, 128).transpose(0, 2, 4, 3, 1)
    x = np.ascontiguousarray(x)                          # [NIB, NJG, 128, JP, IB] f32
    outb = np.empty(x.shape, dtype=np.uint16)
    for ib in range(NIB):
        for jg in range(NJG):
            p = paths[ib * NJG + jg]
            blk = x[ib, jg]
            if p in ("A", "G"):
                v = np.exp(blk).astype(ml_dtypes.bfloat16)
                outb[ib, jg] = v.view(np.uint16)
            elif p == "Y":
                v = (S_LOG2 * blk).astype(ml_dtypes.bfloat16)
                outb[ib, jg] = v.view(np.uint16)
            else:  # S
                v = np.rint(S_LOG2 * blk + C_SCH).astype(np.int16)
                outb[ib, jg] = v.view(np.uint16)
    # [NIB, NJG, 128, JP, IB] -> [NIB, 2, 128, NJG/2, JP, IB]
    outb = outb.reshape(NIB, 2, NJG // 2, 128, JP, IB).transpose(0, 1, 3, 2, 4, 5)
    return np.ascontiguousarray(outb).view(ml_dtypes.bfloat16)


def make_in_maps(q, k, v, pos_bias, Wq, Wk, Wv, Wo):
    q = np.asarray(q, dtype=np.float32)
    k = np.asarray(k, dtype=np.float32)
    v = np.asarray(v, dtype=np.float32)
    pos_bias = np.asarray(pos_bias, dtype=np.float32)
    Wq = np.asarray(Wq, dtype=np.float32)
    Wk = np.asarray(Wk, dtype=np.float32)
    Wv = np.asarray(Wv, dtype=np.float32)
    Wo = np.asarray(Wo, dtype=np.float32)

    paths = tile_paths()
    qkvT = np.ascontiguousarray(
        np.stack([_dn_layout(q), _dn_layout(k), _dn_layout(v)], axis=3)
    )
    ident = np.eye(128, dtype=np.float32).astype(ml_dtypes.bfloat16)

    in_maps = []
    for h in range(NCORES):
        hs = slice(h * DH, (h + 1) * DH)
        woe = np.zeros((DH + 1, D + 1), dtype=np.float32)
        woe[0:DH, 0:D] = Wo[:, hs].T
        woe[DH, D] = 1.0
        in_maps.append({
            "qkvT": qkvT,
            "wq": _w_layout(SCALE * S_LOG2 * Wq[hs, :], 4),
            "wk": _w_layout(Wk[hs, :], 4),
            "wv": _w_layout(Wv[hs, :], 1),
            "wo": np.ascontiguousarray(woe.astype(ml_dtypes.bfloat16)),
            "identT": ident,
            "eb": _eb_layout(pos_bias[h], paths),
        })
    return in_maps


def kernel(q, k, v, pos_bias, Wq, Wk, Wv, Wo):
    nc = _get_nc()
    in_maps = make_in_maps(q, k, v, pos_bias, Wq, Wk, Wv, Wo)
    res = run_bass_kernel_spmd(nc, in_maps, core_ids=list(range(NCORES)))
    acc = None
    for c in range(NCORES):
        o = res.results[c]["out"].astype(np.float32)
        acc = o if acc is None else acc + o
    # [B, NIB, 128p, 4s, 256] -> [B, N, D] with row i = ib*IB + s*JC + p
    return np.ascontiguousarray(
        acc.transpose(0, 1, 3, 2, 4).reshape(B, N, D)
    )
